# revision 58
# baseline (speedup 1.0000x reference)
"""Trainium2 Bass kernel for nn_AxialBlock (3-axis axial attention sum).

Problem (hardcoded): x (B=4, C=512, T=16, H=32, W=32) fp32, three axial
MHA blocks (attend along W, H, T; n_head=8, d=64) each with their own
QKVO projections; outputs summed. Output (B, C, T, H, W) fp32.

Sharding: 8 cores = (batch b in 0..3) x (H-half j in 0..1). Every pass is
computed fully locally (no collectives):
  - w-pass / t-pass: tokens (t, h in owned half, w), fully local.
  - h-pass: attention along H needs all H, so the full batch sample is
    recomputed on both cores of a pair; each core keeps only its owned
    H-half of the output. (For odd cores the H axis is rotated host-side
    so the owned half is always h-positions 0..15 — attention along H is
    permutation-equivariant, so this is exact.)

On-device layout trick: x is channels-first, i.e. already "x^T" (C on
partitions) which is what the PE wants for the QKV projections. The host
pre-permutes x into three token orders (w-fastest / t-fastest / h-fastest)
so that each axial attention acts on 32 consecutive tokens ("rows").

Per 512-token tile (16 rows x 32 tokens):
  q^T, k^T (feat-partition) and v (token-partition) projections in bf16,
  per-(row, head) 32x32 attention scores via PE array tiling (K=64 mode),
  softmax along free dim (exp on ScalarE, per-block reduce on VectorE),
  A -> A^T via the DVE 32x32 stream transpose, o^T = V^T A^T via PE
  (32x64 tiling, writes o^T feat-partition directly), out-projection,
  and accumulation of the three passes through DRAM read-modify-write.

t-axis has seq len 16: two t-fibers are packed into one 32-token row and
a 0/1 block mask zeroes cross-fiber attention after exp.
"""

import contextlib

import ml_dtypes
import numpy as np

import concourse.bass as bass
import concourse.tile as tile
from concourse import bacc, mybir
from concourse.bass_utils import run_bass_kernel_spmd

BF16 = mybir.dt.bfloat16
FP32 = mybir.dt.float32
BF16_NP = np.dtype(ml_dtypes.bfloat16)

B, C, T, H, W = 4, 512, 16, 32, 32
NH, D = 8, 64
HL = H // 2              # per-core H slice
N_CORES = 8
TOK_LOCAL = T * HL * W   # 8192 tokens owned per core
TOK_FULL = T * H * W     # 16384 tokens in a batch sample
TILE = 512               # tokens per on-chip tile
NCH = C // 128           # 4 partition chunks of the feature dim

# dev knob: cap tiles per pass (None = full problem). Truncated builds are
# only for fast AP/scheduling smoke tests - output is wrong when set.
NTILES_CAP = None
# dev knob: repeat the whole workload K times (device-time measurement by
# wall-clock slope; output stays correct since the final repetition's y
# rmw chain overwrites/accumulates identically... NOTE: output is WRONG for
# REPS > 1 (w-pass re-init is fine but t/h re-add; use only for timing).
REPS = 1
# dev knob: ablations for HW time attribution (output wrong when set):
#   "attn"    - skip S matmuls, softmax and O matmuls (out-proj reads v)
#   "softmax" - keep S and O matmuls, skip the softmax/transpose chain
ABLATE = None


def _build_pass(tc, pools, axis, x_ap, w_aps, y_ap, bias_aps, tmask_sb,
                kz_tiles, abd_tiles):
    """Emit one axial-attention pass.

    axis: 'w' | 't' | 'h'.  x_ap: (512, ntok) bf16 DRAM, token order chosen
    so each 32-token group is one attention row.  y_ap: (512, 8192) fp32
    DRAM output accumulator (natural (t, h_local, w) token order).
    """
    nc = tc.nc
    wq_sb, wk_sb, wv_sb, wo_sb = w_aps
    ntok = TOK_FULL if axis == "h" else TOK_LOCAL
    ntiles = ntok // TILE
    if NTILES_CAP is not None:
        ntiles = min(ntiles, NTILES_CAP)

    (xt_pool, qk_pool, v_pool, a_pool, sm_pool,
     ot_pool, y_pool, ps_pool, sps_pool) = pools

    # y viewed (c, t, hl, w) for the strided rmw accumulation
    y4d = y_ap.rearrange("c (t h w) -> c t h w", t=T, h=HL, w=W)

    for it in range(ntiles):
        # ---- load x^T tile: (128, NCH, TILE) bf16, free = (chunk, token)
        xt = xt_pool.tile([128, NCH, TILE], BF16)
        for kc in range(NCH):
            nc.sync.dma_start(
                xt[:, kc, :], x_ap[128 * kc:128 * (kc + 1), it * TILE:(it + 1) * TILE]
            )

        # ---- q^T, k^T projections: feat-partition bf16.
        # h-pass: q is only needed for the owned h-half of each 32-token row
        # (packed to 16 cols per row, N=256); k stays full.
        # k is evacuated parity-split straight into the persistent pre-zeroed
        # kz buffers (head p's 64 d-rows in place, other 64 rows zero), so
        # the S matmul can contract over all 128 partitions — the only legal
        # PE tile positions are row 0 / col 0 (see module docstring).
        qw = TILE // 2 if axis == "h" else TILE
        q_sb = qk_pool.tile([128, NCH, qw], BF16, tag="q")
        kz_sb = kz_tiles[tc._kz_flip]
        tc._kz_flip ^= 1
        for w_sb, nw, ev in ((wq_sb, qw, 0), (wk_sb, TILE, 1)):
            for mc in range(NCH):
                ps = ps_pool.tile([128, TILE], FP32, tag="ps", bufs=2)
                for kc in range(NCH):
                    if nw == TILE:
                        rhs = xt[:, kc, :]
                    else:
                        rhs = xt[:, kc, :].rearrange(
                            "p (a b) -> p a b", a=16)[:, :, 0:HL]
                    nc.tensor.matmul(
                        ps[0:128, 0:nw],
                        lhsT=w_sb[:, kc, 128 * mc:128 * (mc + 1)],
                        rhs=rhs,
                        start=(kc == 0), stop=(kc == NCH - 1),
                    )
                if ev == 0:
                    nc.scalar.copy(q_sb[:, mc, :], ps[0:128, 0:nw])
                elif mc < 2:
                    nc.scalar.copy(kz_sb[0:64, 0, mc, :], ps[0:64, :])
                    nc.scalar.copy(kz_sb[64:128, 1, mc, :], ps[64:128, :])
                else:
                    nc.vector.tensor_copy(kz_sb[0:64, 0, mc, :], ps[0:64, :])
                    nc.vector.tensor_copy(kz_sb[64:128, 1, mc, :], ps[64:128, :])

        # ---- v projection, token-partition: (128, NCH, C) bf16,
        #      free = (token block ts, feature)
        v_sb = v_pool.tile([128, NCH, C], BF16)
        for ts in range(NCH):
            ps = ps_pool.tile([128, TILE], FP32, tag="ps", bufs=2)
            for kc in range(NCH):
                nc.tensor.matmul(
                    ps[:],
                    lhsT=xt[:, kc, 128 * ts:128 * (ts + 1)],
                    rhs=wv_sb[:, kc, :],
                    start=(kc == 0), stop=(kc == NCH - 1),
                )
            if ts % 2 == 0:
                nc.scalar.copy(v_sb[:, ts, :], ps[:])
            else:
                nc.vector.tensor_copy(v_sb[:, ts, :], ps[:])

        # ---- attention: 16 rows x 8 heads of 32x32 blocks
        otw = TILE // 2 if axis == "h" else TILE
        qm = 16 if axis == "h" else 32   # query rows kept per 32-token row
        GW = NH * 32                     # 256 free columns per row group
        if ABLATE != "attn":
            # ---- scores + softmax at 2-rowgroup granularity: S psum
            # (128, 512) = one bank; free = (g%2)*256 + head-slot*32 + kpos.
            # One matmul per (chunk, row) computes BOTH heads of the chunk:
            # the moving operand stacks kz[par=0] and kz[par=1] columns
            # (N=64), sharing a single q stationary load.
            for gg in range(2):
                sps = sps_pool.tile([128, 2 * GW], FP32)
                if axis == "h":
                    # unowned query rows stay unwritten by the S matmuls;
                    # zero them so the softmax reads defined values
                    nc.vector.memset(sps[:], 0.0)
                for gh in range(2):
                    g = 2 * gg + gh
                    for c in range(NCH):
                        for j in range(4):
                            qcol = (g * 4 + j) * qm
                            nc.tensor.matmul(
                                sps[32 * j:32 * j + qm,
                                    gh * GW + 2 * c * 32:gh * GW + (2 * c + 2) * 32],
                                lhsT=q_sb[:, c, qcol:qcol + qm],
                                rhs=kz_sb[:, :, c,
                                          (g * 4 + j) * 32:(g * 4 + j) * 32 + 32],
                                tile_position=(0, 32 * j),
                            )
                if ABLATE == "softmax":
                    continue
                # ---- softmax along k, one op per step per 2 row groups
                a_sb = a_pool.tile([128, 2 * GW], BF16, tag="a")
                nc.scalar.activation(a_sb[:], sps[:],
                                     mybir.ActivationFunctionType.Exp)
                a3 = a_sb[:].rearrange("p (n k) -> p n k", n=2 * NH)
                if axis == "t":
                    nc.vector.tensor_tensor(
                        a3, a3,
                        tmask_sb[:].unsqueeze(1).broadcast_to((128, 2 * NH, 32)),
                        mybir.AluOpType.mult,
                    )
                sums = sm_pool.tile([128, 2 * NH], FP32, tag="sums")
                nc.vector.tensor_reduce(
                    sums[:], a3, axis=mybir.AxisListType.X,
                    op=mybir.AluOpType.add
                )
                recip = sm_pool.tile([128, 2 * NH], FP32, tag="recip")
                nc.vector.reciprocal(recip[:], sums[:])
                nc.vector.tensor_tensor(
                    a3, a3,
                    recip[:].unsqueeze(2).broadcast_to((128, 2 * NH, 32)),
                    mybir.AluOpType.mult,
                )
                # A -> A^T in place (DVE 32x32 block transpose, full width),
                # then GpSimd scatters each (g, row j) slab to its diagonal
                # slot in the per-rowgroup block-diagonal a_bd buffers (other
                # partitions stay zero from the one-time memset).
                at_sb = a_pool.tile([128, 2 * GW], BF16, tag="at")
                nc.vector.transpose(at_sb[:], a_sb[:])
                for gh in range(2):
                    g = 2 * gg + gh
                    for j in range(4):
                        nc.gpsimd.tensor_copy(
                            abd_tiles[g][32 * j:32 * (j + 1),
                                         256 * j:256 * (j + 1)],
                            at_sb[32 * j:32 * (j + 1), gh * GW:(gh + 1) * GW],
                        )

        # ---- o^T = V^T A_bd, chunk-outer so only one o^T psum bank is live
        # at a time (the persistent abd buffers hold all 4 row groups);
        # evacuate each chunk to bf16 SBUF as soon as it completes
        ot_sb = ot_pool.tile([128, NCH, otw], BF16)
        if ABLATE == "attn":
            for c in range(NCH):
                nc.gpsimd.tensor_copy(ot_sb[:, c, :], v_sb[:, c, 0:otw])
        else:
            abd4s = [abd_tiles[g][:].rearrange("p (j n q) -> p j n q",
                                               j=4, n=NH) for g in range(4)]
            for c in range(NCH):
                otp = ps_pool.tile([128, otw], FP32, name="otp", tag="otp",
                                   bufs=2)
                for g in range(4):
                    for p in range(2):
                        nc.tensor.matmul(
                            otp[64 * p:64 * (p + 1),
                                g * 4 * qm:(g + 1) * 4 * qm],
                            lhsT=v_sb[:, g,
                                      (2 * c + p) * 64:(2 * c + p + 1) * 64],
                            rhs=abd4s[g][:, :, 2 * c + p, 0:qm],
                            tile_position=(0, 64 * p),
                        )
                if c % 2 == 0:
                    nc.scalar.copy(ot_sb[:, c, :], otp[:])
                else:
                    nc.vector.tensor_copy(ot_sb[:, c, :], otp[:])

        # ---- out-projection + accumulate into y (h-pass: owned tokens only)
        for mc in range(NCH):
            yps = ps_pool.tile([128, otw], FP32, name="yps", tag="yps", bufs=2)
            for kc in range(NCH):
                nc.tensor.matmul(
                    yps[:],
                    lhsT=wo_sb[:, kc, 128 * mc:128 * (mc + 1)],
                    rhs=ot_sb[:, kc, :],
                    start=(kc == 0), stop=(kc == NCH - 1),
                )
            cs = slice(128 * mc, 128 * (mc + 1))
            if axis == "w":
                # first pass: plain write, fold the (summed) output bias in
                y_sb = y_pool.tile([128, TILE], FP32, tag="yw")
                nc.scalar.activation(
                    y_sb[:], yps[:], mybir.ActivationFunctionType.Identity,
                    bias=bias_aps[mc],
                )
                nc.sync.dma_start(y_ap[cs, it * TILE:(it + 1) * TILE], y_sb[:])
            elif axis == "t":
                # tile it covers h-row `it`; psum tokens are (w 32, t 16)
                # t-fastest, DRAM side stays natural (t-major, w contiguous)
                y_slice = y4d[cs, :, it, :]                       # (128, t16, w32)
                yprev = y_pool.tile([128, T, W], FP32, tag="yt")
                nc.sync.dma_start(yprev[:], y_slice)
                ynew = y_pool.tile([128, T, W], FP32, tag="yt2")
                yp3 = yps[:].rearrange("p (w t) -> p w t", w=W).transpose([0, 2, 1])
                nc.vector.tensor_tensor(
                    ynew[:], yprev[:], yp3, mybir.AluOpType.add
                )
                nc.sync.dma_start(y_slice, ynew[:])
            else:
                # h-pass: tile it covers t = it//2, w-half = it%2, tokens
                # (tw 16, h 32) h-fastest; owned h is always positions 0..15
                t_idx, w_half = it // 2, it % 2
                ws = slice(16 * w_half, 16 * (w_half + 1))
                y_slice = y4d[cs, t_idx, :, ws]                   # (128, hl16, w16)
                yprev = y_pool.tile([128, HL, 16], FP32, tag="yh")
                nc.sync.dma_start(yprev[:], y_slice)
                ynew = y_pool.tile([128, HL, 16], FP32, tag="yh2")
                yp3 = (yps[:].rearrange("p (w h) -> p w h", w=16)
                       .transpose([0, 2, 1]))
                nc.vector.tensor_tensor(
                    ynew[:], yprev[:], yp3, mybir.AluOpType.add
                )
                nc.sync.dma_start(y_slice, ynew[:])


def build_program():
    """Build + compile the SPMD bass program (same program on all 8 cores)."""
    nc = bacc.Bacc(
        "TRN2", target_bir_lowering=False, debug=False,
        enable_asserts=False, num_devices=N_CORES,
    )

    def din(name, shape, dt=BF16):
        return nc.dram_tensor(name, shape, dt, kind="ExternalInput").ap()

    x_w = din("x_w", (C, TOK_LOCAL))
    x_t = din("x_t", (C, TOK_LOCAL))
    x_h = din("x_h", (C, TOK_FULL))
    w_in = {}
    for ax in ("w", "t", "h"):
        for nm in ("wq", "wk", "wv", "wo"):
            w_in[f"{nm}_{ax}"] = din(f"{nm}_{ax}", (C, C))
    bias_in = din("bias", (C, 1), FP32)
    tmask_in = din("tmask", (128, 32))

    y_ap = nc.dram_tensor("y", (C, TOK_LOCAL), FP32, kind="ExternalOutput").ap()

    with tile.TileContext(nc) as tc:
        with contextlib.ExitStack() as ctx:
            xt_pool = ctx.enter_context(tc.tile_pool(name="xt", bufs=3))
            w_pool = ctx.enter_context(tc.tile_pool(name="wts", bufs=2))
            qk_pool = ctx.enter_context(tc.tile_pool(name="qk", bufs=2))
            v_pool = ctx.enter_context(tc.tile_pool(name="v", bufs=2))
            a_pool = ctx.enter_context(tc.tile_pool(name="a", bufs=3))
            sm_pool = ctx.enter_context(tc.tile_pool(name="sm", bufs=3))
            ot_pool = ctx.enter_context(tc.tile_pool(name="ot", bufs=2))
            y_pool = ctx.enter_context(tc.tile_pool(name="y", bufs=3))
            ps_pool = ctx.enter_context(tc.tile_pool(name="ps", bufs=2, space="PSUM"))
            sps_pool = ctx.enter_context(tc.tile_pool(name="sps", bufs=2, space="PSUM"))
            const_pool = ctx.enter_context(tc.tile_pool(name="const", bufs=1))

            # constants
            tmask_sb = const_pool.tile([128, 32], BF16)
            nc.sync.dma_start(tmask_sb[:], tmask_in[:])
            bias_sb = const_pool.tile([128, NCH], FP32)
            for mc in range(NCH):
                nc.sync.dma_start(
                    bias_sb[:, mc:mc + 1], bias_in[128 * mc:128 * (mc + 1), :]
                )
            bias_aps = [bias_sb[:, mc:mc + 1] for mc in range(NCH)]

            # persistent block-diagonal A^T buffers (one per row group for
            # decoupled pipelining) and parity-split k buffers, zeroed once
            abd_tiles = []
            for i in range(4):
                t = const_pool.tile([128, 4 * NH * 32], BF16, name=f"abd{i}")
                nc.gpsimd.memset(t[:], 0.0)
                abd_tiles.append(t)
            kz_tiles = []
            for i in range(2):
                t = const_pool.tile([128, 2, NCH, TILE], BF16, name=f"kz{i}")
                nc.gpsimd.memset(t[:], 0.0)
                kz_tiles.append(t)
            tc._kz_flip = 0

            pools = (xt_pool, qk_pool, v_pool, a_pool, sm_pool,
                     ot_pool, y_pool, ps_pool, sps_pool)

            for _rep in range(REPS):
              for ax, x_ap in (("w", x_w), ("t", x_t), ("h", x_h)):
                w_aps = []
                for nm in ("wq", "wk", "wv", "wo"):
                    wt = w_pool.tile([128, NCH, C], BF16, tag=nm, name=nm)
                    for kc in range(NCH):
                        nc.sync.dma_start(
                            wt[:, kc, :],
                            w_in[f"{nm}_{ax}"][128 * kc:128 * (kc + 1), :],
                        )
                    w_aps.append(wt)
                _build_pass(tc, pools, ax, x_ap, w_aps, y_ap, bias_aps, tmask_sb,
                            kz_tiles, abd_tiles)

    nc.compile()
    return nc


_PROGRAM = None


def _get_program():
    global _PROGRAM
    if _PROGRAM is None:
        _PROGRAM = build_program()
    return _PROGRAM


def make_in_maps(inputs):
    """Host-side shard + layout prep: per-core input dicts."""
    x = np.asarray(inputs["x"], np.float32)          # (B, C, T, H, W)
    scale = 1.0 / np.sqrt(D)

    weights = {}
    for ax in ("w", "h", "t"):
        for nm in ("wq", "wk", "wv", "wo"):
            wm = np.asarray(inputs[f"{nm}_{ax}"], np.float32)
            if nm == "wq":
                wm = wm * scale
            # lhsT layout: (C_in, C_out) = W.T
            weights[f"{nm}_{ax}"] = np.ascontiguousarray(wm.T).astype(BF16_NP)
    bias = (np.asarray(inputs["bo_w"], np.float32)
            + np.asarray(inputs["bo_h"], np.float32)
            + np.asarray(inputs["bo_t"], np.float32)).reshape(C, 1)

    # t-pass cross-fiber 0/1 mask: partitions = 4 row-blocks x 32 qpos,
    # free = 32 kpos; two 16-long t-fibers per 32-token row.
    p = np.arange(128) % 32
    k = np.arange(32)
    tmask = ((p[:, None] // 16) == (k[None, :] // 16)).astype(BF16_NP)

    in_maps = []
    for core in range(N_CORES):
        b, j = divmod(core, 2)
        xb = x[b]                                    # (C, T, H, W)
        xw = xb[:, :, 16 * j:16 * (j + 1), :]        # (C, T, HL, W) w-fastest
        xt = np.transpose(xw, (0, 2, 3, 1))          # (C, HL, W, T) t-fastest
        xh = np.transpose(xb, (0, 1, 3, 2))          # (C, T, W, H) h-fastest
        if j == 1:
            # rotate H so the owned half is always h-positions 0..15
            xh = np.concatenate([xh[..., 16:], xh[..., :16]], axis=-1)
        m = {
            "x_w": np.ascontiguousarray(xw).reshape(C, TOK_LOCAL).astype(BF16_NP),
            "x_t": np.ascontiguousarray(xt).reshape(C, TOK_LOCAL).astype(BF16_NP),
            "x_h": np.ascontiguousarray(xh).reshape(C, TOK_FULL).astype(BF16_NP),
            "bias": bias, "tmask": tmask,
        }
        m.update(weights)
        in_maps.append(m)
    return in_maps


def assemble_output(results):
    """Gather per-core y (C, 8192) into (B, C, T, H, W) fp32."""
    out = np.empty((B, C, T, H, W), np.float32)
    for core in range(N_CORES):
        b, j = divmod(core, 2)
        y = np.asarray(results[core]["y"]).reshape(C, T, HL, W)
        out[b, :, :, 16 * j:16 * (j + 1), :] = y
    return out


def kernel(**inputs) -> np.ndarray:
    nc = _get_program()
    in_maps = make_in_maps(inputs)
    res = run_bass_kernel_spmd(nc, in_maps, core_ids=list(range(N_CORES)))
    return assemble_output(res.results)


# revision 70
# speedup vs baseline: 66.0365x; 66.0365x over previous
"""Trainium2 Bass kernel for nn_AxialBlock (3-axis axial attention sum).

Problem (hardcoded): x (B=4, C=512, T=16, H=32, W=32) fp32, three axial
MHA blocks (attend along W, H, T; n_head=8, d=64) each with their own
QKVO projections; outputs summed. Output (B, C, T, H, W) fp32.

Sharding: 8 cores = (batch b in 0..3) x (H-half j in 0..1). Every pass is
computed fully locally (no collectives):
  - w-pass / t-pass: tokens (t, h in owned half, w), fully local.
  - h-pass: attention along H needs all H, so the full batch sample is
    recomputed on both cores of a pair; each core keeps only its owned
    H-half of the output. (For odd cores the H axis is rotated host-side
    so the owned half is always h-positions 0..15 — attention along H is
    permutation-equivariant, so this is exact.)

On-device layout trick: x is channels-first, i.e. already "x^T" (C on
partitions) which is what the PE wants for the QKV projections. The host
pre-permutes x into three token orders (w-fastest / t-fastest / h-fastest)
so that each axial attention acts on 32 consecutive tokens ("rows").

Per 512-token tile (16 rows x 32 tokens):
  q^T (feat-partition) and v (token-partition) projections in bf16; k is
  evacuated parity-split into persistent pre-zeroed "kz" buffers (one head
  per 64 d-rows, rest zero) so attention scores can contract over all 128
  partitions - the PE array tiling positions with BOTH row!=0 and col!=0
  hard-crash the device (NRT_EXEC_UNIT_UNRECOVERABLE), so only (0, col) /
  (row, 0) tiles are usable. Scores: one (K=128, M=32, N=64) matmul per
  (chunk, row) computing both heads of the chunk at col-tile (0, 32j).
  Softmax is batched per 2 row groups with one op per step: exp on
  ScalarE, per-block reduce + reciprocal on VectorE, and the broadcast
  normalize on GpSimd (measured 4x faster there than on VectorE). The
  t-pass cross-fiber mask is a rank-2 matmul (-60 additive) accumulated
  under the scores before exp; the h-pass zero-fill of unowned query rows
  is a rank-1 zero matmul (PSUM accumulation base trick).
  A -> A^T via the full-width DVE 32x32 block transpose, then per-row
  contiguous (32, 512) DVE copies form a block-diagonal A^T ("abd") in
  persistent zeroed double buffers; o^T = V^T @ abd lands feat-partition
  directly as one (K=128, M=64, N=4*qm) matmul per (chunk, head); then the
  out-projection, and accumulation of the three passes into y through
  DRAM read-modify-write (w writes + bias, t/h strided rmw adds).

t-axis has seq len 16: two t-fibers are packed into one 32-token row with
the rank-2 mask zeroing cross-fiber attention. h-pass computes only the
owned half of the queries (qm=16 per row, packed), halving its q
projection, scores, o^T and out-projection work.
"""

import contextlib

import ml_dtypes
import numpy as np

import concourse.bass as bass
import concourse.tile as tile
from concourse import bacc, mybir
from concourse.bass_utils import run_bass_kernel_spmd

BF16 = mybir.dt.bfloat16
FP32 = mybir.dt.float32
BF16_NP = np.dtype(ml_dtypes.bfloat16)

B, C, T, H, W = 4, 512, 16, 32, 32
NH, D = 8, 64
HL = H // 2              # per-core H slice
N_CORES = 8
TOK_LOCAL = T * HL * W   # 8192 tokens owned per core
TOK_FULL = T * H * W     # 16384 tokens in a batch sample
TILE = 512               # tokens per on-chip tile
NCH = C // 128           # 4 partition chunks of the feature dim

# dev knob: cap tiles per pass (None = full problem). Truncated builds are
# only for fast AP/scheduling smoke tests - output is wrong when set.
NTILES_CAP = None
# dev knob: repeat the whole workload K times (device-time measurement by
# wall-clock slope; output stays correct since the final repetition's y
# rmw chain overwrites/accumulates identically... NOTE: output is WRONG for
# REPS > 1 (w-pass re-init is fine but t/h re-add; use only for timing).
REPS = 1
# dev knob: ablations for HW time attribution (output wrong when set):
#   "attn"    - skip S matmuls, softmax and O matmuls (out-proj reads v)
#   "softmax" - keep S and O matmuls, skip the softmax/transpose chain
ABLATE = None


def _build_pass(tc, pools, axis, x_ap, w_aps, y_ap, bias_aps, tml_sb, tmr_sb,
                kz_tiles, abd_tiles):
    """Emit one axial-attention pass.

    axis: 'w' | 't' | 'h'.  x_ap: (512, ntok) bf16 DRAM, token order chosen
    so each 32-token group is one attention row.  y_ap: (512, 8192) fp32
    DRAM output accumulator (natural (t, h_local, w) token order).
    """
    nc = tc.nc
    wq_sb, wk_sb, wv_sb, wo_sb = w_aps
    ntok = TOK_FULL if axis == "h" else TOK_LOCAL
    ntiles = ntok // TILE
    if NTILES_CAP is not None:
        ntiles = min(ntiles, NTILES_CAP)

    (xt_pool, qk_pool, v_pool, a_pool, sm_pool,
     ot_pool, y_pool, ps_pool, sps_pool) = pools

    # y viewed (c, t, hl, w) for the strided rmw accumulation
    y4d = y_ap.rearrange("c (t h w) -> c t h w", t=T, h=HL, w=W)

    for it in range(ntiles):
        # ---- load x^T tile: (128, NCH, TILE) bf16, free = (chunk, token)
        xt = xt_pool.tile([128, NCH, TILE], BF16)
        for kc in range(NCH):
            nc.sync.dma_start(
                xt[:, kc, :], x_ap[128 * kc:128 * (kc + 1), it * TILE:(it + 1) * TILE]
            )

        # ---- q^T, k^T projections: feat-partition bf16.
        # h-pass: q is only needed for the owned h-half of each 32-token row
        # (packed to 16 cols per row, N=256); k stays full.
        # k is evacuated parity-split straight into the persistent pre-zeroed
        # kz buffers (head p's 64 d-rows in place, other 64 rows zero), so
        # the S matmul can contract over all 128 partitions — the only legal
        # PE tile positions are row 0 / col 0 (see module docstring).
        qw = TILE // 2 if axis == "h" else TILE
        q_sb = qk_pool.tile([128, NCH, qw], BF16, tag="q")
        kz_sb = kz_tiles[tc._kz_flip]
        tc._kz_flip ^= 1
        for w_sb, nw, ev in ((wq_sb, qw, 0), (wk_sb, TILE, 1)):
            for mc in range(NCH):
                ps = ps_pool.tile([128, TILE], FP32, tag="ps", bufs=2)
                for kc in range(NCH):
                    if nw == TILE:
                        rhs = xt[:, kc, :]
                    else:
                        rhs = xt[:, kc, :].rearrange(
                            "p (a b) -> p a b", a=16)[:, :, 0:HL]
                    nc.tensor.matmul(
                        ps[0:128, 0:nw],
                        lhsT=w_sb[:, kc, 128 * mc:128 * (mc + 1)],
                        rhs=rhs,
                        start=(kc == 0), stop=(kc == NCH - 1),
                    )
                if ev == 0:
                    nc.scalar.copy(q_sb[:, mc, :], ps[0:128, 0:nw])
                elif mc < 2:
                    nc.scalar.copy(kz_sb[0:64, 0, mc, :], ps[0:64, :])
                    nc.scalar.copy(kz_sb[64:128, 1, mc, :], ps[64:128, :])
                else:
                    nc.vector.tensor_copy(kz_sb[0:64, 0, mc, :], ps[0:64, :])
                    nc.vector.tensor_copy(kz_sb[64:128, 1, mc, :], ps[64:128, :])

        # ---- v projection, token-partition: (128, NCH, C) bf16,
        #      free = (token block ts, feature)
        v_sb = v_pool.tile([128, NCH, C], BF16)
        for ts in range(NCH):
            ps = ps_pool.tile([128, TILE], FP32, tag="ps", bufs=2)
            for kc in range(NCH):
                nc.tensor.matmul(
                    ps[:],
                    lhsT=xt[:, kc, 128 * ts:128 * (ts + 1)],
                    rhs=wv_sb[:, kc, :],
                    start=(kc == 0), stop=(kc == NCH - 1),
                )
            if ts % 2 == 0:
                nc.scalar.copy(v_sb[:, ts, :], ps[:])
            else:
                nc.vector.tensor_copy(v_sb[:, ts, :], ps[:])

        # ---- attention: 16 rows x 8 heads of 32x32 blocks
        otw = TILE // 2 if axis == "h" else TILE
        qm = 16 if axis == "h" else 32   # query rows kept per 32-token row
        GW = NH * 32                     # 256 free columns per row group
        abd_by_g = {}
        if ABLATE != "attn":
            # ---- scores + softmax at 2-rowgroup granularity: S psum
            # (128, 512) = one bank; free = (g%2)*256 + head-slot*32 + kpos.
            # One matmul per (chunk, row) computes BOTH heads of the chunk:
            # the moving operand stacks kz[par=0] and kz[par=1] columns
            # (N=64), sharing a single q stationary load.
            for gg in range(2):
                sps = sps_pool.tile([128, 2 * GW], FP32)
                # Base matmul written FIRST with start=True over the full
                # width; the S matmuls then accumulate onto it.
                #   t: rank-2 additive cross-fiber mask (-60 off-fiber, so
                #      exp kills those entries and the reduce needs no mask)
                #   h: rank-1 zeros (unowned query rows are never written by
                #      the S matmuls but are read by the full-width softmax)
                #   w: none needed - S matmuls run standalone (start=True)
                base = axis != "w"
                if axis == "t":
                    nc.tensor.matmul(
                        sps[:], lhsT=tml_sb[:], rhs=tmr_sb[:],
                        start=True, stop=False, skip_group_check=True,
                    )
                elif axis == "h":
                    nc.tensor.matmul(
                        sps[:], lhsT=tc._z_sb[:, 0:128], rhs=tc._z_sb[:],
                        start=True, stop=False, skip_group_check=True,
                    )
                nmm = 32
                i_mm = 0
                for gh in range(2):
                    g = 2 * gg + gh
                    for c in range(NCH):
                        for j in range(4):
                            qcol = (g * 4 + j) * qm
                            i_mm += 1
                            nc.tensor.matmul(
                                sps[32 * j:32 * j + qm,
                                    gh * GW + 2 * c * 32:gh * GW + (2 * c + 2) * 32],
                                lhsT=q_sb[:, c, qcol:qcol + qm],
                                rhs=kz_sb[:, :, c,
                                          (g * 4 + j) * 32:(g * 4 + j) * 32 + 32],
                                tile_position=(0, 32 * j),
                                start=(not base),
                                stop=(base and i_mm == nmm),
                                skip_group_check=True,
                            )
                if ABLATE == "softmax":
                    abd_by_g[2 * gg] = abd_tiles[gg % 2]
                    abd_by_g[2 * gg + 1] = abd_tiles[gg % 2]
                    continue
                # ---- softmax along k, one op per step per 2 row groups
                a_sb = a_pool.tile([128, 2 * GW], BF16, tag="a")
                nc.scalar.activation(a_sb[:], sps[:],
                                     mybir.ActivationFunctionType.Exp)
                a3 = a_sb[:].rearrange("p (n k) -> p n k", n=2 * NH)
                sums = sm_pool.tile([128, 2 * NH], FP32, tag="sums")
                nc.vector.tensor_reduce(
                    sums[:], a3, axis=mybir.AxisListType.X,
                    op=mybir.AluOpType.add
                )
                recip = sm_pool.tile([128, 2 * NH], FP32, tag="recip")
                nc.vector.reciprocal(recip[:], sums[:])
                # normalize on GpSimd (measured ~4x faster than DVE for the
                # broadcast multiply), freeing VectorE for the transposes
                nc.gpsimd.tensor_tensor(
                    a3, a3,
                    recip[:].unsqueeze(2).broadcast_to((128, 2 * NH, 32)),
                    mybir.AluOpType.mult,
                )
                # A -> A^T in place (DVE 32x32 block transpose, full width),
                # then per-row contiguous DVE copies into the block-diagonal
                # a_bd buffer: columns are row-major (j*512 + gh*256 +
                # head*32 + q), so each (j) slab is one (32, 512) copy.
                # Off-diagonal partitions stay zero from the one-time memset.
                at_sb = a_pool.tile([128, 2 * GW], BF16, tag="at")
                nc.vector.transpose(at_sb[:], a_sb[:])
                abd = abd_tiles[tc._abd_flip]
                tc._abd_flip ^= 1
                for j in range(4):
                    nc.vector.tensor_copy(
                        abd[32 * j:32 * (j + 1), 512 * j:512 * (j + 1)],
                        at_sb[32 * j:32 * (j + 1), :],
                    )
                abd_by_g[2 * gg] = abd
                abd_by_g[2 * gg + 1] = abd

        # ---- o^T = V^T A_bd, chunk-outer so only one o^T psum bank is live
        # at a time (the persistent abd buffers hold all 4 row groups);
        # evacuate each chunk to bf16 SBUF as soon as it completes
        ot_sb = ot_pool.tile([128, NCH, otw], BF16)
        if ABLATE == "attn":
            for c in range(NCH):
                nc.gpsimd.tensor_copy(ot_sb[:, c, :], v_sb[:, c, 0:otw])
        else:
            for c in range(NCH):
                otp = ps_pool.tile([128, otw], FP32, name="otp", tag="otp",
                                   bufs=2)
                for g in range(4):
                    gh = g % 2
                    abd4 = abd_by_g[g][:].rearrange("p (j x) -> p j x", j=4)
                    for p in range(2):
                        s0 = gh * GW + (2 * c + p) * 32
                        nc.tensor.matmul(
                            otp[64 * p:64 * (p + 1),
                                g * 4 * qm:(g + 1) * 4 * qm],
                            lhsT=v_sb[:, g,
                                      (2 * c + p) * 64:(2 * c + p + 1) * 64],
                            rhs=abd4[:, :, s0:s0 + qm],
                            tile_position=(0, 64 * p),
                        )
                if c % 2 == 0:
                    nc.scalar.copy(ot_sb[:, c, :], otp[:])
                else:
                    nc.vector.tensor_copy(ot_sb[:, c, :], otp[:])

        # ---- out-projection + accumulate into y (h-pass: owned tokens only)
        for mc in range(NCH):
            yps = ps_pool.tile([128, otw], FP32, name="yps", tag="yps", bufs=2)
            for kc in range(NCH):
                nc.tensor.matmul(
                    yps[:],
                    lhsT=wo_sb[:, kc, 128 * mc:128 * (mc + 1)],
                    rhs=ot_sb[:, kc, :],
                    start=(kc == 0), stop=(kc == NCH - 1),
                )
            cs = slice(128 * mc, 128 * (mc + 1))
            if axis == "w":
                # first pass: plain write, fold the (summed) output bias in
                y_sb = y_pool.tile([128, TILE], FP32, tag="yw")
                nc.scalar.activation(
                    y_sb[:], yps[:], mybir.ActivationFunctionType.Identity,
                    bias=bias_aps[mc],
                )
                nc.sync.dma_start(y_ap[cs, it * TILE:(it + 1) * TILE], y_sb[:])
            elif axis == "t":
                # tile it covers h-row `it`; psum tokens are (w 32, t 16)
                # t-fastest, DRAM side stays natural (t-major, w contiguous)
                y_slice = y4d[cs, :, it, :]                       # (128, t16, w32)
                yprev = y_pool.tile([128, T, W], FP32, tag="yt")
                nc.sync.dma_start(yprev[:], y_slice)
                ynew = y_pool.tile([128, T, W], FP32, tag="yt2")
                yp3 = yps[:].rearrange("p (w t) -> p w t", w=W).transpose([0, 2, 1])
                nc.vector.tensor_tensor(
                    ynew[:], yprev[:], yp3, mybir.AluOpType.add
                )
                nc.sync.dma_start(y_slice, ynew[:])
            else:
                # h-pass: tile it covers t = it//2, w-half = it%2, tokens
                # (tw 16, h 32) h-fastest; owned h is always positions 0..15
                t_idx, w_half = it // 2, it % 2
                ws = slice(16 * w_half, 16 * (w_half + 1))
                y_slice = y4d[cs, t_idx, :, ws]                   # (128, hl16, w16)
                yprev = y_pool.tile([128, HL, 16], FP32, tag="yh")
                nc.sync.dma_start(yprev[:], y_slice)
                ynew = y_pool.tile([128, HL, 16], FP32, tag="yh2")
                yp3 = (yps[:].rearrange("p (w h) -> p w h", w=16)
                       .transpose([0, 2, 1]))
                nc.vector.tensor_tensor(
                    ynew[:], yprev[:], yp3, mybir.AluOpType.add
                )
                nc.sync.dma_start(y_slice, ynew[:])


def build_program():
    """Build + compile the SPMD bass program (same program on all 8 cores)."""
    nc = bacc.Bacc(
        "TRN2", target_bir_lowering=False, debug=False,
        enable_asserts=False, num_devices=N_CORES,
    )

    def din(name, shape, dt=BF16):
        return nc.dram_tensor(name, shape, dt, kind="ExternalInput").ap()

    x_w = din("x_w", (C, TOK_LOCAL))
    x_t = din("x_t", (C, TOK_LOCAL))
    x_h = din("x_h", (C, TOK_FULL))
    w_in = {}
    for ax in ("w", "t", "h"):
        for nm in ("wq", "wk", "wv", "wo"):
            w_in[f"{nm}_{ax}"] = din(f"{nm}_{ax}", (C, C))
    bias_in = din("bias", (C, 1), FP32)
    tml_in = din("tml", (2, 128))
    tmr_in = din("tmr", (2, 512))

    y_ap = nc.dram_tensor("y", (C, TOK_LOCAL), FP32, kind="ExternalOutput").ap()

    with tile.TileContext(nc) as tc:
        with contextlib.ExitStack() as ctx:
            xt_pool = ctx.enter_context(tc.tile_pool(name="xt", bufs=3))
            w_pool = ctx.enter_context(tc.tile_pool(name="wts", bufs=2))
            qk_pool = ctx.enter_context(tc.tile_pool(name="qk", bufs=2))
            v_pool = ctx.enter_context(tc.tile_pool(name="v", bufs=2))
            a_pool = ctx.enter_context(tc.tile_pool(name="a", bufs=3))
            sm_pool = ctx.enter_context(tc.tile_pool(name="sm", bufs=3))
            ot_pool = ctx.enter_context(tc.tile_pool(name="ot", bufs=2))
            y_pool = ctx.enter_context(tc.tile_pool(name="y", bufs=3))
            ps_pool = ctx.enter_context(tc.tile_pool(name="ps", bufs=2, space="PSUM"))
            sps_pool = ctx.enter_context(tc.tile_pool(name="sps", bufs=2, space="PSUM"))
            const_pool = ctx.enter_context(tc.tile_pool(name="const", bufs=1))

            # constants
            tml_sb = const_pool.tile([2, 128], BF16)
            nc.sync.dma_start(tml_sb[:], tml_in[:])
            tmr_sb = const_pool.tile([2, 512], BF16)
            nc.sync.dma_start(tmr_sb[:], tmr_in[:])
            z_sb = const_pool.tile([1, 512], BF16)
            nc.gpsimd.memset(z_sb[:], 0.0)
            tc._z_sb = z_sb
            bias_sb = const_pool.tile([128, NCH], FP32)
            for mc in range(NCH):
                nc.sync.dma_start(
                    bias_sb[:, mc:mc + 1], bias_in[128 * mc:128 * (mc + 1), :]
                )
            bias_aps = [bias_sb[:, mc:mc + 1] for mc in range(NCH)]

            # persistent block-diagonal A^T buffers (double-buffered per
            # 2-rowgroup softmax) and parity-split k buffers, zeroed once
            abd_tiles = []
            for i in range(2):
                t = const_pool.tile([128, 4 * 512], BF16, name=f"abd{i}")
                nc.gpsimd.memset(t[:], 0.0)
                abd_tiles.append(t)
            tc._abd_flip = 0
            kz_tiles = []
            for i in range(2):
                t = const_pool.tile([128, 2, NCH, TILE], BF16, name=f"kz{i}")
                nc.gpsimd.memset(t[:], 0.0)
                kz_tiles.append(t)
            tc._kz_flip = 0

            pools = (xt_pool, qk_pool, v_pool, a_pool, sm_pool,
                     ot_pool, y_pool, ps_pool, sps_pool)

            for _rep in range(REPS):
              for ax, x_ap in (("w", x_w), ("t", x_t), ("h", x_h)):
                w_aps = []
                for nm in ("wq", "wk", "wv", "wo"):
                    wt = w_pool.tile([128, NCH, C], BF16, tag=nm, name=nm)
                    for kc in range(NCH):
                        nc.sync.dma_start(
                            wt[:, kc, :],
                            w_in[f"{nm}_{ax}"][128 * kc:128 * (kc + 1), :],
                        )
                    w_aps.append(wt)
                _build_pass(tc, pools, ax, x_ap, w_aps, y_ap, bias_aps, tml_sb,
                            tmr_sb, kz_tiles, abd_tiles)

    nc.compile()
    return nc


_PROGRAM = None


def _get_program():
    global _PROGRAM
    if _PROGRAM is None:
        _PROGRAM = build_program()
    return _PROGRAM


def make_in_maps(inputs):
    """Host-side shard + layout prep: per-core input dicts."""
    x = np.asarray(inputs["x"], np.float32)          # (B, C, T, H, W)
    scale = 1.0 / np.sqrt(D)

    weights = {}
    for ax in ("w", "h", "t"):
        for nm in ("wq", "wk", "wv", "wo"):
            wm = np.asarray(inputs[f"{nm}_{ax}"], np.float32)
            if nm == "wq":
                wm = wm * scale
            # lhsT layout: (C_in, C_out) = W.T
            weights[f"{nm}_{ax}"] = np.ascontiguousarray(wm.T).astype(BF16_NP)
    bias = (np.asarray(inputs["bo_w"], np.float32)
            + np.asarray(inputs["bo_h"], np.float32)
            + np.asarray(inputs["bo_t"], np.float32)).reshape(C, 1)

    # t-pass cross-fiber 0/1 mask: partitions = 4 row-blocks x 32 qpos,
    # free = 32 kpos; two 16-long t-fibers per 32-token row.
    # rank-2 additive cross-fiber mask for the t-pass:
    # S += tml.T @ tmr with tml one-hot on the query fiber and tmr = -60 on
    # cross-fiber key columns
    p = np.arange(128) % 32
    tml = np.stack([(p // 16) == e for e in range(2)]).astype(BF16_NP)
    f = np.arange(512) % 32
    tmr = np.stack([np.where((f // 16) != e, -60.0, 0.0) for e in range(2)]
                   ).astype(BF16_NP)

    in_maps = []
    for core in range(N_CORES):
        b, j = divmod(core, 2)
        xb = x[b]                                    # (C, T, H, W)
        xw = xb[:, :, 16 * j:16 * (j + 1), :]        # (C, T, HL, W) w-fastest
        xt = np.transpose(xw, (0, 2, 3, 1))          # (C, HL, W, T) t-fastest
        xh = np.transpose(xb, (0, 1, 3, 2))          # (C, T, W, H) h-fastest
        if j == 1:
            # rotate H so the owned half is always h-positions 0..15
            xh = np.concatenate([xh[..., 16:], xh[..., :16]], axis=-1)
        m = {
            "x_w": np.ascontiguousarray(xw).reshape(C, TOK_LOCAL).astype(BF16_NP),
            "x_t": np.ascontiguousarray(xt).reshape(C, TOK_LOCAL).astype(BF16_NP),
            "x_h": np.ascontiguousarray(xh).reshape(C, TOK_FULL).astype(BF16_NP),
            "bias": bias, "tml": tml, "tmr": tmr,
        }
        m.update(weights)
        in_maps.append(m)
    return in_maps


def assemble_output(results):
    """Gather per-core y (C, 8192) into (B, C, T, H, W) fp32."""
    out = np.empty((B, C, T, H, W), np.float32)
    for core in range(N_CORES):
        b, j = divmod(core, 2)
        y = np.asarray(results[core]["y"]).reshape(C, T, HL, W)
        out[b, :, :, 16 * j:16 * (j + 1), :] = y
    return out


def kernel(**inputs) -> np.ndarray:
    nc = _get_program()
    in_maps = make_in_maps(inputs)
    res = run_bass_kernel_spmd(nc, in_maps, core_ids=list(range(N_CORES)))
    return assemble_output(res.results)
